# revision 26
# baseline (speedup 1.0000x reference)
"""Trainium2 Bass kernel for nn_EssentialMatrixEstimator (v2).

Distribution (8 cores):
  - XN: natural row-shard  (384 rows x 3072 cols) -> exact row top-3 thresholds.
  - XC: transposed col-shard (384 cols x 3072 rows as [col, row]) -> exact col
    top-3 thresholds + dense masking + col-sharded gram.
  - coll1: AllGather of per-core row thresholds (384 f32 -> 3072).
  - coll2: AllReduce of the 6x6 gram C' on PRE-CENTERED monomials.

Math: the (N*M,9) epipolar Gram collapses to the 6x6 monomial Gram C'.
Monomials are pre-centered about the host constant c0 (grid centroid), so C'
is well-conditioned; the Hartley normalization is recovered from C' moments
(row/col 5) and applied as a 6x6 L-transform C2 = L1 C' L2^T instead of a
second gram pass.  Mmat (9x9) is an index expansion of C2; min-eigvector via
50-step shifted power iteration (rescaled repeated squaring), projection via
a 32-step 6x6 blockdiag chain (insensitive; validated 2.9e-4).

The big T = M2'^T W^T contraction streams in float32r (1 cy/row); validated
tolerant to tf32/bf16-level rounding (5e-4 / 3.9e-3 final rel err).  T chunks
are PE-transposed into TT [128, (j b)] (partition-scattered DMA transposes
measured 15-20us and were replaced).  The tail runs gpsimd-free (PE ones-
matmul broadcasts/reductions) to avoid gpsimd library-swap stalls; w9
normalization is deferred and folded into the final output scale.
"""

import os

os.environ.setdefault("JAX_PLATFORMS", "axon")

import numpy as np

import concourse.bass as bass
import concourse.bass_isa as bass_isa
import concourse.mybir as mybir
import concourse.bacc as bacc
import concourse.tile as tile

NCORES = 8
N = 3072
SH = N // NCORES          # 384 rows/cols per core
RT = SH // 128            # 3 tiles per core shard
CB = N // 128             # 24 tiles across the full dim
F32 = mybir.dt.float32
F32R = mybir.dt.float32r
AF = mybir.ActivationFunctionType
OP = mybir.AluOpType
AX = mybir.AxisListType

EPS = 1e-8
SQRT2 = 1.4142135623730951
INV_SQRT3 = 1.0 / 1.7320508075688772
T0 = float(np.nextafter(np.float32(0.01), np.float32(1)))  # x > 0.01 == x >= T0
H, W = 64, 64

# cpack const layout (tensor [9, C_TOT]): column ranges
C_I9H = 0      # I9 * 0.5            [9, 9]
C_ET69 = 9     # E^T selector        [6, 9]
C_I3 = 18      # I3                  [3, 3]
C_V09 = 21     # full(1/3)           [9, 1]
C_V06 = 22     # full(1/sqrt3)       [6, 1]
C_SEL1 = 23    # [I3 | 0]            [3, 6]
C_SEL2 = 29    # [0 | I3]            [3, 6]
C_SHT = 35     # Sh component mats^T: I6, E1^T..E5^T   [6, 6*6]
C_MSK = 71     # svec masks [c2m c1m c0m]  [6, 3]
C_IDN = 74     # identity 9x9        [9, 9]
C_ONE = 83     # all-ones            [9, 9]
C_TOT = 92

PAIRS = [(0, 0), (0, 1), (0, 2), (1, 1), (1, 2), (2, 2)]


def _pidx():
    d = {}
    for i, (a, b) in enumerate(PAIRS):
        d[(a, b)] = i
        d[(b, a)] = i
    return d


def grid_pts(K):
    idx = np.arange(H * W, dtype=np.float32)
    pix = np.stack([idx % np.float32(W), np.floor(idx / np.float32(W))], -1)
    K_inv = np.linalg.inv(np.asarray(K, np.float32)).astype(np.float32)
    p1h = np.concatenate([pix[:N], np.ones((N, 1), np.float32)], -1)
    pts = (p1h @ K_inv.T)[:, :2].astype(np.float32)
    return pts


def host_constants(K):
    """Pre-centered monomials + packed tail constants (f32)."""
    pts = grid_pts(K)
    x, y = pts[:, 0], pts[:, 1]
    c0x = np.float32(x.mean())
    c0y = np.float32(y.mean())
    xs = (x - c0x).astype(np.float32)
    ys = (y - c0y).astype(np.float32)
    Mp = np.stack([xs * xs, xs * ys, xs, ys * ys, ys, np.ones_like(xs)],
                  -1).astype(np.float32)

    cpack = np.zeros((9, C_TOT), np.float32)
    cpack[:9, C_I9H:C_I9H + 9] = 0.5 * np.eye(9, dtype=np.float32)
    pid = _pidx()
    for a in range(3):
        for b in range(3):
            cpack[pid[(a, b)], C_ET69 + 3 * a + b] = 1.0
    cpack[:3, C_I3:C_I3 + 3] = np.eye(3, dtype=np.float32)
    cpack[:9, C_V09] = 1.0 / 3.0
    cpack[:6, C_V06] = INV_SQRT3
    cpack[:3, C_SEL1:C_SEL1 + 3] = np.eye(3, dtype=np.float32)
    cpack[:3, C_SEL2 + 3:C_SEL2 + 6] = np.eye(3, dtype=np.float32)

    # Sh(dx,dy) = I + dx*E1 + dy*E2 + dx^2*E3 + dx*dy*E4 + dy^2*E5
    # (rows of L before the diag scale; see proto.Lmat)
    E1 = np.zeros((6, 6), np.float32)  # dx terms
    E1[0, 2] = -2.0
    E1[1, 4] = -1.0
    E1[2, 5] = -1.0
    E2 = np.zeros((6, 6), np.float32)  # dy terms
    E2[1, 2] = -1.0
    E2[3, 4] = -2.0
    E2[4, 5] = -1.0
    E3 = np.zeros((6, 6), np.float32)  # dx^2
    E3[0, 5] = 1.0
    E4 = np.zeros((6, 6), np.float32)  # dx*dy
    E4[1, 5] = 1.0
    E5 = np.zeros((6, 6), np.float32)  # dy^2
    E5[3, 5] = 1.0
    mats = [np.eye(6, dtype=np.float32), E1, E2, E3, E4, E5]
    for i, Em in enumerate(mats):
        cpack[:6, C_SHT + 6 * i:C_SHT + 6 * i + 6] = Em.T
    # svec masks: svec = [s2,s2,s,s2,s,1] = c2m*s2 + c1m*s + c0m
    cpack[:6, C_MSK + 0] = [1, 1, 0, 1, 0, 0]
    cpack[:6, C_MSK + 1] = [0, 0, 1, 0, 1, 0]
    cpack[:6, C_MSK + 2] = [0, 0, 0, 0, 0, 1]
    cpack[:9, C_IDN:C_IDN + 9] = np.eye(9, dtype=np.float32)
    cpack[:9, C_ONE:C_ONE + 9] = 1.0
    return Mp, cpack, float(c0x), float(c0y)


def _tile128(a, ntiles):
    """[ntiles*128, F] -> [128, ntiles*F] with [p, t*F+f] = a[t*128+p, f]."""
    F = a.shape[1]
    return np.ascontiguousarray(
        a.reshape(ntiles, 128, F).transpose(1, 0, 2).reshape(128, ntiles * F)
    )


DEFAULT_K = np.array([[500.0, 0.0, 320.0], [0.0, 500.0, 240.0],
                      [0.0, 0.0, 1.0]], np.float32)


def build_nc(repeats=1, no_coll=False, no_tail=False, use_f32r=True,
             dbg_c=False, c0=None):
    if c0 is None:
        _, _, c0x_, c0y_ = host_constants(DEFAULT_K)
        c0 = (c0x_, c0y_)
    nc = bacc.Bacc("TRN2", target_bir_lowering=False, debug=False,
                   num_devices=NCORES)

    xn = nc.dram_tensor("xn", [128, RT * N], F32, kind="ExternalInput")
    xc = nc.dram_tensor("xc", [128, RT * N], F32, kind="ExternalInput")
    m1f = nc.dram_tensor("m1f", [128, CB * 6], F32, kind="ExternalInput")
    m2s = nc.dram_tensor("m2s", [128, RT * 6], F32, kind="ExternalInput")
    cpk = nc.dram_tensor("cpack", [9, C_TOT], F32, kind="ExternalInput")
    out_d = nc.dram_tensor("out", [6, 6] if dbg_c else [3, 3], F32, kind="ExternalOutput")

    tr_in = nc.dram_tensor("tr_in", [1, SH], F32)
    tr_out = nc.dram_tensor("tr_out", [NCORES, SH], F32, addr_space="Shared")
    cr_in = nc.dram_tensor("cr_in", [6, 6], F32)
    cr_out = nc.dram_tensor("cr_out", [6, 6], F32, addr_space="Shared")
    stage = nc.dram_tensor("stage", [64], F32)
    mshuf = nc.dram_tensor("mshuf", [81], F32)

    groups = [list(range(NCORES))]

    with tile.TileContext(nc) as tc:
        with (
            tc.tile_pool(name="persist", bufs=1) as pp,
            tc.tile_pool(name="scratch", bufs=2) as sp,
            tc.tile_pool(name="ps_t", bufs=2, space="PSUM") as ps,
            tc.tile_pool(name="ps_T", bufs=2, space="PSUM") as psT,
            tc.tile_pool(name="ps_c", bufs=1, space="PSUM") as psc,
        ):
            for _rep in range(repeats):
                # ---------- P0: loads (XN on qSP, XC on qACT) ----------
                XN = pp.tile([128, RT * N], F32, tag="XN")
                XC = pp.tile([128, RT * N], F32, tag="XC")
                HN = N // 2
                for t in range(RT):
                    a = t * N
                    nc.sync.dma_start(XN[:, a:a + HN], xn[:, a:a + HN])
                    nc.scalar.dma_start(XN[:, a + HN:a + N],
                                        xn[:, a + HN:a + N])
                for t in range(RT):
                    a = t * N
                    nc.sync.dma_start(XC[:, a:a + HN], xc[:, a:a + HN])
                    nc.scalar.dma_start(XC[:, a + HN:a + N],
                                        xc[:, a + HN:a + N])
                m1s_s = pp.tile([128, CB * 6], F32, tag="m1f")
                nc.scalar.dma_start(m1s_s[:], m1f[:])
                m2s_s = pp.tile([128, RT * 6], F32, tag="m2s")
                nc.scalar.dma_start(m2s_s[:], m2s[:])
                cps = pp.tile([9, C_TOT], F32, tag="cpk")
                nc.scalar.dma_start(cps[:], cpk[:])
                sqwarm = sp.tile([1, 1], F32, tag="sqwarm")
                nc.scalar.activation(sqwarm[:], cps[0:1, 0:1], AF.Sqrt)

                def XNt(t):
                    return XN[:, t * N:(t + 1) * N]

                def XCt(t):
                    return XC[:, t * N:(t + 1) * N]

                # ---------- P1: row thresholds -> coll1 ----------
                r8 = pp.tile([128, RT * 8], F32, tag="r8")
                for t in range(RT):
                    nc.vector.max(out=r8[:, t * 8:t * 8 + 8], in_=XNt(t))
                trT0 = pp.tile([128, RT], F32, tag="trT0")
                nc.vector.tensor_scalar_max(
                    trT0[:],
                    r8[:].rearrange("p (t e) -> p t e", e=8)[:, :, 2], T0)
                for t in range(RT):
                    nc.sync.dma_start(tr_in[0:1, t * 128:(t + 1) * 128],
                                      trT0[:, t:t + 1])

                if no_coll:
                    nc.sync.dma_start(tr_out[0:1, :], tr_in[:])
                else:
                    nc.gpsimd.collective_compute(
                        "AllGather", OP.bypass, replica_groups=groups,
                        ins=[tr_in[:]], outs=[tr_out[:]])

                # ---------- P2: col thresholds (local, exact) ----------
                c8 = pp.tile([128, RT * 8], F32, tag="c8")
                for t in range(RT):
                    nc.vector.max(out=c8[:, t * 8:t * 8 + 8], in_=XCt(t))

                # ---------- P3: broadcast row-threshold table ----------
                trow = pp.tile([1, N], F32, tag="trow")
                nc.sync.dma_start(trow[:], tr_out[:].rearrange("k i -> (k i)"))
                trB = pp.tile([128, N], F32, tag="trB")
                MCH = 1536
                for c0_, cw in ((0, 768), (768, 768), (1536, 1536)):
                    nc.gpsimd.partition_broadcast(
                        trB[:, c0_:c0_ + cw], trow[:, c0_:c0_ + cw],
                        channels=128)

                # ---------- P4: dense mask + fp32r T-gram ----------
                # W (f32r): W = XC * [XC >= max(trB, tc_t)]
                # T[b, r] = sum_c m2'[c, b] * W^T[c, r]   (PSUM chunks [6,512])
                WDT = F32R if use_f32r else F32
                m2r = pp.tile([128, RT * 6], WDT, tag="m2r")
                nc.vector.tensor_copy(m2r[:], m2s_s[:])
                Wr = pp.tile([128, RT * N], WDT, tag="Wr")
                Wf = pp.tile([128, N], F32, tag="Wf")  # t=2 chunk via gpsimd
                Tsb = pp.tile([6, N], F32, tag="Tsb")
                TT = pp.tile([128, CB * 6], F32, tag="TT")
                i6 = cps[0:6, C_IDN:C_IDN + 6]
                for h in range(2):
                    for t in range(RT):
                        tcl = c8[:, t * 8 + 2:t * 8 + 3]
                        sl = slice(t * N + h * MCH, t * N + (h + 1) * MCH)
                        msk = pp.tile([128, MCH], F32, tag=f"msk{h}{t}")
                        nc.vector.scalar_tensor_tensor(
                            msk[:], trB[:, h * MCH:(h + 1) * MCH], tcl,
                            XC[:, sl], OP.max, OP.is_le)
                        if t == 0:
                            nc.gpsimd.tensor_tensor(
                                Wf[:, h * MCH:(h + 1) * MCH], XC[:, sl],
                                msk[:], OP.mult)
                        else:
                            nc.vector.tensor_tensor(Wr[:, sl], XC[:, sl],
                                                    msk[:], OP.mult)
                    for q in range(3):
                        ch = h * 3 + q
                        Tp = psT.tile([6, 512], F32, tag="Tp")
                        for t in range(RT):
                            c0_ = t * N + h * MCH + q * 512
                            if t == 0:
                                nc.tensor.matmul(
                                    Tp[:], m2s_s[:, t * 6:(t + 1) * 6],
                                    Wf[:, h * MCH + q * 512:
                                        h * MCH + q * 512 + 512],
                                    start=True, stop=False)
                            else:
                                nc.tensor.matmul(
                                    Tp[:],
                                    m2r[:, t * 6:(t + 1) * 6],
                                    Wr[:, c0_:c0_ + 512],
                                    start=False, stop=(t == RT - 1))
                        nc.scalar.activation(Tsb[:, ch * 512:(ch + 1) * 512],
                                             Tp[:], AF.Copy)
                        # PE-transpose T chunk into TT[p, (j b)] blocks
                        for jj in range(4):
                            j = ch * 4 + jj
                            pt = ps.tile([128, 6], F32, tag="tps")
                            nc.tensor.transpose(
                                pt[:], Tsb[:, j * 128:(j + 1) * 128], i6)
                            nc.scalar.activation(TT[:, j * 6:(j + 1) * 6],
                                                 pt[:], AF.Copy)

                # C[a, b] = sum_j m1'_j^T TT_j  (two groups for overlap)
                pc0 = psc.tile([6, 6], F32, tag="pc0")
                pc1 = psc.tile([6, 6], F32, tag="pc1")
                for j in range(CB):
                    pc = pc0 if j < 12 else pc1
                    nc.tensor.matmul(pc[:], m1s_s[:, j * 6:(j + 1) * 6],
                                     TT[:, j * 6:(j + 1) * 6],
                                     start=(j % 12 == 0), stop=(j % 12 == 11))
                Cp = sp.tile([6, 6], F32, tag="Cp")
                nc.vector.tensor_copy(Cp[:], pc0[:])
                nc.vector.tensor_tensor(Cp[:], Cp[:], pc1[:], OP.add)
                nc.sync.dma_start(cr_in[:], Cp[:])

                # ---------- coll2: AllReduce 6x6 gram ----------
                if no_coll:
                    nc.sync.dma_start(cr_out[:], cr_in[:])
                else:
                    nc.gpsimd.collective_compute(
                        "AllReduce", OP.add, replica_groups=groups,
                        ins=[cr_in[:]], outs=[cr_out[:]])

                if no_tail:
                    nn = 6 if dbg_c else 3
                    dummy = sp.tile([nn, nn], F32, tag="dummy")
                    nc.sync.dma_start(dummy[:], cr_out[0:nn, 0:nn])
                    nc.sync.dma_start(out_d[:], dummy[:])
                    continue

                # ---------- tail ----------
                _tail(nc, pp, sp, ps, psc, cps, c0, cr_out, stage, mshuf, out_d)

    nc.compile()
    return nc


def _transpose(nc, ps, sp, in_sb, n, idn, tag):
    pt = ps.tile([n, n], F32, tag="tps")
    nc.tensor.transpose(pt[:], in_sb, idn[:n, :n])
    ot = sp.tile([n, n], F32, tag=f"ot_{tag}")
    nc.vector.tensor_copy(ot[:], pt[:])
    return ot


def _powchain(nc, ps, sp, m_sb, n, tag, n_squarings=5, extra=True):
    """M^50 (extra=True: 5 squarings + M48=M32@M16 + M50=M48@M2) or M^32."""
    powers = {}
    cur = m_sb
    for i in range(1, n_squarings + 1):
        pm = ps.tile([n, n], F32, tag="tps")
        nc.tensor.matmul(pm[:], cur, cur, start=True, stop=True)
        nxt = sp.tile([n, n], F32, tag=f"pw_{tag}_{i}")
        nc.vector.tensor_scalar_mul(nxt[:], pm[:], 2.0)
        powers[2 ** i] = nxt
        cur = nxt[:]
    if not extra:
        return powers[2 ** n_squarings]
    pm = ps.tile([n, n], F32, tag="tps")
    nc.tensor.matmul(pm[:], powers[32][:], powers[16][:], start=True, stop=True)
    m48 = sp.tile([n, n], F32, tag=f"pw_{tag}_48")
    nc.vector.tensor_scalar_mul(m48[:], pm[:], 2.0)
    pm = ps.tile([n, n], F32, tag="tps")
    nc.tensor.matmul(pm[:], m48[:], powers[2][:], start=True, stop=True)
    m50 = sp.tile([n, n], F32, tag=f"pw_{tag}_50")
    nc.vector.tensor_scalar_mul(m50[:], pm[:], 2.0)
    return m50


def _tail(nc, pp, sp, ps, psc, cps, c0, cr_out, stage, mshuf, out_d):
    """C' -> Hartley -> L-transform -> Mmat -> chains -> projection."""
    idn = cps[0:9, C_IDN:C_IDN + 9]

    Cp = sp.tile([6, 6], F32, tag="Cpr")
    nc.sync.dma_start(Cp[:], cr_out[:])
    CpT = sp.tile([6, 6], F32, tag="CprT")
    nc.scalar.dma_start(CpT[:], cr_out[:].rearrange("a b -> b a"))

    # moments [1,12] extracted to partition 0 via e5-selector matmuls
    # (side1 = C'[:,5] = e5^T CpT ; side2 = C'[5,:] = e5^T Cp) -- no DMAs
    sc = pp.tile([128, 112], F32, tag="tailsc")
    e5 = cps[0:6, C_IDN + 5:C_IDN + 6]
    mo1 = ps.tile([1, 6], F32, tag="tps")
    nc.tensor.matmul(mo1[:], e5, CpT[:], start=True, stop=True)
    mo2 = ps.tile([1, 6], F32, tag="tps")
    nc.tensor.matmul(mo2[:], e5, Cp[:], start=True, stop=True)
    nc.vector.tensor_copy(sc[0:1, 0:6], mo1[:])
    nc.vector.tensor_copy(sc[0:1, 6:12], mo2[:])

    def scv(a, b):
        return sc[0:1, a:b]

    def pair(k):
        return sc[0:1, 0:12].rearrange("p (g d) -> p d g", g=2)[:, k, :]

    Sxx, Sx, Syy, Sy, Sw = pair(0), pair(2), pair(3), pair(4), pair(5)
    ws = scv(12, 14); nc.vector.tensor_scalar_add(ws, Sw, EPS)
    rws = scv(14, 16); nc.vector.reciprocal(rws, ws)
    cx = scv(16, 18); nc.vector.tensor_tensor(cx, Sx, rws, OP.mult)  # = dx
    cy = scv(18, 20); nc.vector.tensor_tensor(cy, Sy, rws, OP.mult)  # = dy
    t_a = scv(20, 22); nc.vector.tensor_tensor(t_a, cx, Sx, OP.mult)
    t_b = scv(22, 24); nc.vector.tensor_tensor(t_b, cy, Sy, OP.mult)
    cdS = scv(24, 26); nc.vector.tensor_tensor(cdS, t_a, t_b, OP.add)
    u_a = scv(26, 28); nc.vector.tensor_tensor(u_a, cx, cx, OP.mult)
    u_b = scv(28, 30); nc.vector.tensor_tensor(u_b, cy, cy, OP.mult)
    c2_ = scv(30, 32); nc.vector.tensor_tensor(c2_, u_a, u_b, OP.add)
    sq_ = scv(32, 34); nc.vector.tensor_tensor(sq_, Sxx, Syy, OP.add)
    n2c = scv(34, 36); nc.vector.tensor_scalar_mul(n2c, cdS, -2.0)
    c2w = scv(36, 38); nc.vector.tensor_tensor(c2w, c2_, Sw, OP.mult)
    m_ = scv(38, 40); nc.vector.tensor_tensor(m_, sq_, n2c, OP.add)
    m2_ = scv(40, 42); nc.vector.tensor_tensor(m2_, m_, c2w, OP.add)
    md2 = scv(42, 44); nc.vector.tensor_tensor(md2, m2_, rws, OP.mult)
    md2e = scv(44, 46); nc.vector.tensor_scalar_add(md2e, md2, EPS)
    md = scv(46, 48); nc.scalar.activation(md, md2e, AF.Sqrt)
    mde = scv(48, 50); nc.vector.tensor_scalar_add(mde, md, EPS)
    rmd = scv(50, 52); nc.vector.reciprocal(rmd, mde)
    s_ = scv(52, 54); nc.vector.tensor_scalar_mul(s_, rmd, SQRT2)
    # real centroids: cr = dx + c0 ; c0s = [c0x c0x c0y c0y] paired
    cxr = scv(54, 56); nc.vector.tensor_scalar_add(cxr, cx, c0[0])
    cyr = scv(56, 58); nc.vector.tensor_scalar_add(cyr, cy, c0[1])
    scx = scv(58, 60); nc.vector.tensor_tensor(scx, s_, cxr, OP.mult)
    scy = scv(60, 62); nc.vector.tensor_tensor(scy, s_, cyr, OP.mult)
    nscx = scv(62, 64); nc.vector.tensor_scalar_mul(nscx, scx, -1.0)
    nscy = scv(64, 66); nc.vector.tensor_scalar_mul(nscy, scy, -1.0)
    # L scalars: s2, dx2, dxy, dy2 (paired)
    s2p = scv(66, 68); nc.vector.tensor_tensor(s2p, s_, s_, OP.mult)
    dx2 = scv(68, 70); nc.vector.tensor_tensor(dx2, cx, cx, OP.mult)
    dxy = scv(70, 72); nc.vector.tensor_tensor(dxy, cx, cy, OP.mult)
    dy2 = scv(72, 74); nc.vector.tensor_tensor(dy2, cy, cy, OP.mult)

    # T row-major 9-vectors: t1v at 76:85, t2v at 85:94
    nc.vector.memset(scv(76, 94), 0.0)
    tv = sc[0:1, 76:94]
    tv9 = tv.rearrange("p (v f) -> p v f", v=2)
    nc.vector.tensor_copy(tv9[:, :, 0:1], s_.unsqueeze(2))
    nc.vector.tensor_copy(tv9[:, :, 4:5], s_.unsqueeze(2))
    nc.vector.tensor_copy(
        tv9[:, :, 2:8].rearrange("p v (c d) -> p v c d", c=2)[:, :, :, 0:1],
        sc[0:1, 62:66].rearrange("p (c v) -> p v c", c=2).unsqueeze(3))
    nc.vector.memset(tv9[:, :, 8:9], 1.0)
    nc.sync.dma_start(stage[0:18], tv)
    T12 = sp.tile([3, 6], F32, tag="T12")
    nc.sync.dma_start(
        T12[:].rearrange("i (v j) -> i v j", v=2),
        stage[0:18].rearrange("(v i j) -> i v j", i=3, j=3))

    # broadcast scalar strip to 6 partitions for the L build (PE ones);
    # consumers read the PSUM bank directly
    ones16 = cps[0:1, C_ONE:C_ONE + 6]
    scBt = psc.tile([6, 80], F32, tag="scBp")
    nc.tensor.matmul(scBt[:], ones16, sc[0:1, 0:80], start=True, stop=True)
    scB = scBt

    def shT(side, tag):
        """Sh^T for side (0/1): I^T + dx E1^T + dy E2^T + dx2 E3^T + ..."""
        dx = scB[:, 16 + side:17 + side]
        dy = scB[:, 18 + side:19 + side]
        dx2_ = scB[:, 68 + side:69 + side]
        dxy_ = scB[:, 70 + side:71 + side]
        dy2_ = scB[:, 72 + side:73 + side]
        def M(i):
            return cps[0:6, C_SHT + 6 * i:C_SHT + 6 * i + 6]
        acc = sp.tile([6, 6], F32, tag=f"sh_{tag}")
        nc.vector.scalar_tensor_tensor(acc[:], M(1), dx, M(0), OP.mult, OP.add)
        for i, sval in [(2, dy), (3, dx2_), (4, dxy_), (5, dy2_)]:
            nc.vector.scalar_tensor_tensor(acc[:], M(i), sval, acc[:],
                                           OP.mult, OP.add)
        return acc

    Sh1T = shT(0, "1")
    Sh2T = shT(1, "2")
    # svec side1 as a [6,1] column (per-partition): c2m*s2 + c1m*s + c0m
    sv1c = sp.tile([6, 1], F32, tag="sv1c")
    tmp1 = sp.tile([6, 1], F32, tag="svt1")
    nc.vector.scalar_tensor_tensor(
        tmp1[:], cps[0:6, C_MSK:C_MSK + 1], scB[:, 66:67],
        cps[0:6, C_MSK + 2:C_MSK + 3], OP.mult, OP.add)
    nc.vector.scalar_tensor_tensor(
        sv1c[:], cps[0:6, C_MSK + 1:C_MSK + 2], scB[:, 52:53],
        tmp1[:], OP.mult, OP.add)
    # svec side2 as a [1,6] row on partition 0: [s2 s2 s s2 s 1]
    svr2 = sc[0:1, 96:102]
    s2v2 = sc[0:1, 67:68]
    sv2 = sc[0:1, 53:54]
    nc.vector.tensor_copy(
        svr2.rearrange("p (a b) -> p a b", a=3)[:, 0:2, 0:1],
        s2v2.unsqueeze(2).to_broadcast([1, 2, 1]))   # slots 0,2 = s2 (a-major)
    nc.vector.tensor_copy(svr2[:, 1:2], s2v2)        # slot 1 = s2
    nc.vector.tensor_copy(svr2[:, 3:4], s2v2)        # slot 3 = s2
    nc.vector.tensor_copy(svr2[:, 2:3], sv2)         # slot 2 = s
    nc.vector.tensor_copy(svr2[:, 4:5], sv2)         # slot 4 = s
    nc.vector.memset(svr2[:, 5:6], 1.0)
    sv2B = sp.tile([6, 6], F32, tag="sv2B")
    sv2Bp = ps.tile([6, 6], F32, tag="tps")
    nc.tensor.matmul(sv2Bp[:], ones16, svr2, start=True, stop=True)
    nc.vector.tensor_copy(sv2B[:], sv2Bp[:])

    # C2 = D1 Sh1 C' Sh2^T D2
    vps = ps.tile([6, 6], F32, tag="tps")
    nc.tensor.matmul(vps[:], Sh1T[:], Cp[:], start=True, stop=True)  # Sh1 C'
    vS = sp.tile([6, 6], F32, tag="vS")
    nc.vector.tensor_copy(vS[:], vps[:])
    vT = _transpose(nc, ps, sp, vS[:], 6, idn, "vT")
    ups = ps.tile([6, 6], F32, tag="tps")
    nc.tensor.matmul(ups[:], vT[:], Sh2T[:], start=True, stop=True)  # v Sh2^T
    # C2[r, c] = svec1[r] * u[r, c] * svec2[c]
    u1 = sp.tile([6, 6], F32, tag="u1")
    nc.vector.tensor_scalar_mul(u1[:], ups[:], sv1c[:])
    C2 = sp.tile([6, 6], F32, tag="C2")
    nc.vector.tensor_tensor(C2[:], u1[:], sv2B[:], OP.mult)
    C2T = _transpose(nc, ps, sp, C2[:], 6, idn, "c2t")

    _solve(nc, pp, sp, ps, psc, cps, idn, sc, C2[:], C2T[:], stage, mshuf,
           out_d, T12)


def _solve(nc, pp, sp, ps, psc, cps, idn, sc, C2, C2T, stage, mshuf, out_d,
           T12):
    i9h = cps[0:9, C_I9H:C_I9H + 9]
    et69 = cps[0:6, C_ET69:C_ET69 + 9]
    i3c = cps[0:3, C_I3:C_I3 + 3]
    v09 = cps[0:9, C_V09:C_V09 + 1]
    v06 = cps[0:6, C_V06:C_V06 + 1]
    sel1 = cps[0:3, C_SEL1:C_SEL1 + 6]
    sel2 = cps[0:3, C_SEL2:C_SEL2 + 6]

    # G2 = E C2 E^T : G2[3a+b, 3c+d] = C2[pair(a,b), pair(c,d)]
    z_ps = ps.tile([6, 9], F32, tag="tps")
    nc.tensor.matmul(z_ps[:], C2T, et69, start=True, stop=True)  # C2 E^T
    Zs = sp.tile([6, 9], F32, tag="Zs")
    nc.vector.tensor_copy(Zs[:], z_ps[:])
    g_ps = ps.tile([9, 9], F32, tag="tps")
    nc.tensor.matmul(g_ps[:], et69, Zs[:], start=True, stop=True)    # E @ Z
    G2 = sp.tile([9, 9], F32, tag="G2")
    nc.vector.tensor_copy(G2[:], g_ps[:])

    # Mmat[3p+q, 3r+s] = G2[3p+r, 3q+s]: bounce via DRAM
    nc.sync.dma_start(mshuf[:], G2[:])
    Mmat = sp.tile([9, 9], F32, tag="Mmat")
    for p in range(3):
        eng = nc.scalar if p == 1 else nc.sync
        eng.dma_start(
            Mmat[3 * p:3 * p + 3, :].rearrange("q (r s) -> q r s", s=3),
            mshuf[:].rearrange("(p q1 r s) -> p q1 r s", p=3, q1=3, r=3)
            .transpose([0, 2, 1, 3])[p])

    # Msp = Mmat/(2 lam) - I/2
    dg = sp.tile([9, 9], F32, tag="dg")
    nc.vector.tensor_tensor(dg[:], Mmat[:], i9h, OP.mult)
    lam2 = sp.tile([9, 1], F32, tag="lam2")
    nc.vector.tensor_reduce(lam2[:], dg[:], AX.X, OP.add)
    ones99 = cps[0:9, C_ONE:C_ONE + 9]
    lam2r = ps.tile([9, 1], F32, tag="tps")
    nc.tensor.matmul(lam2r[:], ones99, lam2[:], start=True, stop=True)
    lam4 = sp.tile([9, 1], F32, tag="lam4")
    nc.vector.tensor_scalar_mul(lam4[:], lam2r[:], 4.0)
    inv2l = sp.tile([9, 1], F32, tag="inv2l")
    nc.vector.reciprocal(inv2l[:], lam4[:])
    Msp = sp.tile([9, 9], F32, tag="Msp")
    nc.vector.scalar_tensor_tensor(Msp[:], Mmat[:], inv2l[:], i9h,
                                   OP.mult, OP.subtract)
    M50 = _powchain(nc, ps, sp, Msp[:], 9, "m9", 5, extra=True)

    w9ps = ps.tile([1, 9], F32, tag="tps")
    nc.tensor.matmul(w9ps[:], v09, M50[:], start=True, stop=True)
    w9 = sp.tile([1, 9], F32, tag="w9")
    nc.vector.tensor_copy(w9[:], w9ps[:])
    nc.sync.dma_start(stage[24:33], w9[:])  # raw; 1/||w9|| folded at the end
    w9sq = sp.tile([1, 9], F32, tag="w9sq")
    nc.vector.tensor_tensor(w9sq[:], w9[:], w9[:], OP.mult)
    nn9 = sp.tile([1, 1], F32, tag="nn9")
    nc.vector.tensor_reduce(nn9[:], w9sq[:], AX.X, OP.add)
    sr9 = sp.tile([1, 1], F32, tag="sr9")
    nc.scalar.activation(sr9[:], nn9[:], AF.Sqrt)
    rs9 = sp.tile([1, 1], F32, tag="rs9")
    nc.vector.reciprocal(rs9[:], sr9[:])
    rs9c = psc.tile([3, 1], F32, tag="rs9c")
    nc.tensor.matmul(rs9c[:], cps[0:1, C_ONE:C_ONE + 3], rs9[:],
                     start=True, stop=True)

    # E = T2^T E_raw T1 (and E^T);  T1m/T2m preloaded in T12
    T1m = T12[:, 0:3]
    T2m = T12[:, 3:6]
    Eraw = sp.tile([3, 3], F32, tag="Eraw")
    nc.sync.dma_start(Eraw[:], stage[24:33].rearrange("(i j) -> i j", j=3))

    a1ps = ps.tile([3, 3], F32, tag="tps")
    nc.tensor.matmul(a1ps[:], T2m, Eraw[:], start=True, stop=True)
    A1 = sp.tile([3, 3], F32, tag="A1")
    nc.vector.tensor_copy(A1[:], a1ps[:])
    A1T = _transpose(nc, ps, sp, A1[:], 3, idn, "a1t")
    etps = ps.tile([3, 3], F32, tag="tps")
    nc.tensor.matmul(etps[:], T1m, A1T[:], start=True, stop=True)
    ETs = sp.tile([3, 3], F32, tag="ETs")
    nc.vector.tensor_copy(ETs[:], etps[:])
    Es = _transpose(nc, ps, sp, ETs[:], 3, idn, "es")

    # B = E^T E ; blockdiag 6x6 chain (32 iters) for v1 (max) and v3 (min)
    bps = ps.tile([3, 3], F32, tag="tps")
    nc.tensor.matmul(bps[:], Es[:], Es[:], start=True, stop=True)
    Bm = sp.tile([3, 3], F32, tag="Bm")
    nc.vector.tensor_copy(Bm[:], bps[:])
    dg3 = sp.tile([3, 3], F32, tag="dg3")
    nc.vector.tensor_tensor(dg3[:], Bm[:], i3c, OP.mult)
    lb = sp.tile([3, 1], F32, tag="lb")
    nc.vector.tensor_reduce(lb[:], dg3[:], AX.X, OP.add)
    lbr = ps.tile([3, 1], F32, tag="tps")
    nc.tensor.matmul(lbr[:], cps[0:3, C_ONE:C_ONE + 3], lb[:],
                     start=True, stop=True)
    invlb = sp.tile([3, 1], F32, tag="invlb")
    nc.vector.reciprocal(invlb[:], lbr[:])
    Bs3 = sp.tile([3, 3], F32, tag="Bs3")
    nc.vector.tensor_scalar_mul(Bs3[:], Bm[:], invlb[:])
    IB = sp.tile([3, 3], F32, tag="IB")
    nc.vector.tensor_tensor(IB[:], i3c, Bs3[:], OP.subtract)
    bdps = ps.tile([6, 6], F32, tag="tps")
    nc.tensor.matmul(bdps[:, 0:3], sel1, Bs3[:], start=True, stop=True)
    nc.tensor.matmul(bdps[:, 3:6], sel2, IB[:], start=True, stop=True)
    BD = sp.tile([6, 6], F32, tag="BD")
    nc.vector.tensor_copy(BD[:], bdps[:])
    BD32 = _powchain(nc, ps, sp, BD[:], 6, "m6", 5, extra=False)

    w6ps = ps.tile([1, 6], F32, tag="tps")
    nc.tensor.matmul(w6ps[:], v06, BD32[:], start=True, stop=True)
    w6 = sp.tile([1, 6], F32, tag="w6")
    nc.vector.tensor_copy(w6[:], w6ps[:])
    w6sq = sp.tile([1, 6], F32, tag="w6sq")
    nc.vector.tensor_tensor(w6sq[:], w6[:], w6[:], OP.mult)
    nn6 = sp.tile([1, 2], F32, tag="nn6")
    nc.vector.tensor_reduce(nn6[:].unsqueeze(2),
                            w6sq[:].rearrange("p (g d) -> p g d", g=2), AX.X,
                            OP.add)
    sr6 = sp.tile([1, 2], F32, tag="sr6")
    nc.scalar.activation(sr6[:], nn6[:], AF.Sqrt)
    rs6 = sp.tile([1, 2], F32, tag="rs6")
    nc.vector.reciprocal(rs6[:], sr6[:])
    vv = sp.tile([1, 6], F32, tag="vv")
    nc.vector.tensor_tensor(
        vv[:].rearrange("p (g d) -> p g d", g=2),
        w6[:].rearrange("p (g d) -> p g d", g=2),
        rs6[:].unsqueeze(2).to_broadcast([1, 2, 3]), OP.mult)

    # v2 = cross(v3, v1), normalized with EPS
    aa = sp.tile([1, 6], F32, tag="aa")
    nc.vector.tensor_copy(
        aa[:].rearrange("p (r d) -> p r d", r=2),
        vv[:, 3:6].unsqueeze(1).to_broadcast([1, 2, 3]))
    bb = sp.tile([1, 6], F32, tag="bb")
    nc.vector.tensor_copy(
        bb[:].rearrange("p (r d) -> p r d", r=2),
        vv[:, 0:3].unsqueeze(1).to_broadcast([1, 2, 3]))
    cr1 = sp.tile([1, 3], F32, tag="cr1")
    nc.vector.tensor_tensor(cr1[:], aa[:, 1:4], bb[:, 2:5], OP.mult)
    cr2 = sp.tile([1, 3], F32, tag="cr2")
    nc.vector.tensor_tensor(cr2[:], aa[:, 2:5], bb[:, 1:4], OP.mult)
    v2r = sp.tile([1, 3], F32, tag="v2r")
    nc.vector.tensor_tensor(v2r[:], cr1[:], cr2[:], OP.subtract)
    v2sq = sp.tile([1, 3], F32, tag="v2sq")
    nc.vector.tensor_tensor(v2sq[:], v2r[:], v2r[:], OP.mult)
    nn2 = sp.tile([1, 1], F32, tag="nn2")
    nc.vector.tensor_reduce(nn2[:], v2sq[:], AX.X, OP.add)
    sr2 = sp.tile([1, 1], F32, tag="sr2")
    nc.scalar.activation(sr2[:], nn2[:], AF.Sqrt)
    sr2e = sp.tile([1, 1], F32, tag="sr2e")
    nc.vector.tensor_scalar_add(sr2e[:], sr2[:], EPS)
    rs2 = sp.tile([1, 1], F32, tag="rs2")
    nc.vector.reciprocal(rs2[:], sr2e[:])
    v2 = sp.tile([1, 3], F32, tag="v2")
    nc.vector.tensor_tensor(v2[:], v2r[:], rs2[:].to_broadcast([1, 3]), OP.mult)

    vvv = sp.tile([1, 6], F32, tag="vvv")
    nc.vector.tensor_copy(vvv[:, 0:3], vv[:, 0:3])
    nc.vector.tensor_copy(vvv[:, 3:6], v2[:])
    nc.sync.dma_start(stage[33:39], vvv[:])
    Vr = sp.tile([2, 3], F32, tag="Vr")
    nc.sync.dma_start(Vr[:], stage[33:39].rearrange("(i k) -> i k", k=3))
    Vc = sp.tile([3, 2], F32, tag="Vc")
    nc.scalar.dma_start(Vc[:], stage[33:39].rearrange("(i k) -> k i", k=3))
    evps = ps.tile([2, 3], F32, tag="tps")
    nc.tensor.matmul(evps[:], Vc[:], ETs[:], start=True, stop=True)
    Evr = sp.tile([2, 3], F32, tag="Evr")
    nc.vector.tensor_copy(Evr[:], evps[:])
    evsq = sp.tile([2, 3], F32, tag="evsq")
    nc.vector.tensor_tensor(evsq[:], Evr[:], Evr[:], OP.mult)
    ss2 = sp.tile([2, 1], F32, tag="ss2")
    nc.vector.tensor_reduce(ss2[:], evsq[:], AX.X, OP.add)
    sv = sp.tile([2, 1], F32, tag="sv")
    nc.scalar.activation(sv[:], ss2[:], AF.Sqrt)
    ssum = ps.tile([2, 1], F32, tag="tps")
    nc.tensor.matmul(ssum[:], cps[0:2, C_ONE:C_ONE + 2], sv[:],
                     start=True, stop=True)
    savg = sp.tile([2, 1], F32, tag="savg")
    nc.vector.tensor_scalar_mul(savg[:], ssum[:], 0.5)
    sve = sp.tile([2, 1], F32, tag="sve")
    nc.vector.tensor_scalar_add(sve[:], sv[:], EPS)
    rsv = sp.tile([2, 1], F32, tag="rsv")
    nc.vector.reciprocal(rsv[:], sve[:])
    f2 = sp.tile([2, 1], F32, tag="f2")
    nc.vector.tensor_tensor(f2[:], rsv[:], savg[:], OP.mult)
    U2 = sp.tile([2, 3], F32, tag="U2")
    nc.vector.tensor_scalar_mul(U2[:], Evr[:], f2[:])
    ops_ = ps.tile([3, 3], F32, tag="tps")
    nc.tensor.matmul(ops_[:], U2[:], Vr[:], start=True, stop=True)
    outs = sp.tile([3, 3], F32, tag="outs")
    nc.vector.tensor_scalar_mul(outs[:], ops_[:], rs9c[:])
    nc.sync.dma_start(out_d[:], outs[:])


def make_in_maps(P, K):
    P = np.asarray(P, np.float32)
    K = np.asarray(K, np.float32)
    Pc = np.ascontiguousarray(P[:N, :N])
    PcT = np.ascontiguousarray(Pc.T)
    Mp, cpack, c0x, c0y = host_constants(K)
    m1full = _tile128(Mp, CB)
    in_maps = []
    for k in range(NCORES):
        in_maps.append({
            "xn": _tile128(Pc[k * SH:(k + 1) * SH], RT),
            "xc": _tile128(PcT[k * SH:(k + 1) * SH], RT),
            "m1f": m1full,
            "m2s": _tile128(Mp[k * SH:(k + 1) * SH], RT),
            "cpack": cpack,
        })
    return in_maps


_NC_CACHE = {}


def kernel(P, K):
    from concourse.bass_utils import run_bass_kernel_spmd
    if "nc" not in _NC_CACHE:
        _, _, c0x, c0y = host_constants(np.asarray(K, np.float32))
        _NC_CACHE["nc"] = build_nc(c0=(c0x, c0y))
    nc = _NC_CACHE["nc"]
    in_maps = make_in_maps(P, K)
    res = run_bass_kernel_spmd(nc, in_maps, core_ids=list(range(NCORES)))
    return np.asarray(res.results[0]["out"], np.float32)


# revision 27
# speedup vs baseline: 1.0972x; 1.0972x over previous
"""Trainium2 Bass kernel for nn_EssentialMatrixEstimator (v2).

Distribution (8 cores):
  - XN: natural row-shard  (384 rows x 3072 cols) -> exact row top-3 thresholds.
  - XC: transposed col-shard (384 cols x 3072 rows as [col, row]) -> exact col
    top-3 thresholds + dense masking + col-sharded gram.
  - coll1: AllGather of per-core row thresholds (384 f32 -> 3072).
  - coll2: AllReduce of the 6x6 gram C' on PRE-CENTERED monomials.

Math: the (N*M,9) epipolar Gram collapses to the 6x6 monomial Gram C'.
Monomials are pre-centered about the host constant c0 (grid centroid), so C'
is well-conditioned; the Hartley normalization is recovered from C' moments
(row/col 5) and applied as a 6x6 L-transform C2 = L1 C' L2^T instead of a
second gram pass.  Mmat (9x9) is an index expansion of C2; min-eigvector via
50-step shifted power iteration (rescaled repeated squaring), projection via
a 32-step 6x6 blockdiag chain (insensitive; validated 2.9e-4).

The big T = M2'^T W^T contraction streams in float32r (1 cy/row); validated
tolerant to tf32/bf16-level rounding (5e-4 / 3.9e-3 final rel err).  T chunks
are PE-transposed into TT [128, (j b)] (partition-scattered DMA transposes
measured 15-20us and were replaced).  The tail runs gpsimd-free (PE ones-
matmul broadcasts/reductions) to avoid gpsimd library-swap stalls; w9
normalization is deferred and folded into the final output scale.
"""

import os

os.environ.setdefault("JAX_PLATFORMS", "axon")

import numpy as np

import concourse.bass as bass
import concourse.bass_isa as bass_isa
import concourse.mybir as mybir
import concourse.bacc as bacc
import concourse.tile as tile

NCORES = 8
N = 3072
SH = N // NCORES          # 384 rows/cols per core
RT = SH // 128            # 3 tiles per core shard
CB = N // 128             # 24 tiles across the full dim
F32 = mybir.dt.float32
F32R = mybir.dt.float32r
AF = mybir.ActivationFunctionType
OP = mybir.AluOpType
AX = mybir.AxisListType

EPS = 1e-8
SQRT2 = 1.4142135623730951
INV_SQRT3 = 1.0 / 1.7320508075688772
T0 = float(np.nextafter(np.float32(0.01), np.float32(1)))  # x > 0.01 == x >= T0
H, W = 64, 64

# cpack const layout (tensor [9, C_TOT]): column ranges
C_I9H = 0      # I9 * 0.5            [9, 9]
C_ET69 = 9     # E^T selector        [6, 9]
C_I3 = 18      # I3                  [3, 3]
C_V09 = 21     # full(1/3)           [9, 1]
C_V06 = 22     # full(1/sqrt3)       [6, 1]
C_SEL1 = 23    # [I3 | 0]            [3, 6]
C_SEL2 = 29    # [0 | I3]            [3, 6]
C_SHT = 35     # Sh component mats^T: I6, E1^T..E5^T   [6, 6*6]
C_MSK = 71     # svec masks [c2m c1m c0m]  [6, 3]
C_IDN = 74     # identity 9x9        [9, 9]
C_ONE = 83     # all-ones            [9, 9]
C_TOT = 92

PAIRS = [(0, 0), (0, 1), (0, 2), (1, 1), (1, 2), (2, 2)]


def _pidx():
    d = {}
    for i, (a, b) in enumerate(PAIRS):
        d[(a, b)] = i
        d[(b, a)] = i
    return d


def grid_pts(K):
    idx = np.arange(H * W, dtype=np.float32)
    pix = np.stack([idx % np.float32(W), np.floor(idx / np.float32(W))], -1)
    K_inv = np.linalg.inv(np.asarray(K, np.float32)).astype(np.float32)
    p1h = np.concatenate([pix[:N], np.ones((N, 1), np.float32)], -1)
    pts = (p1h @ K_inv.T)[:, :2].astype(np.float32)
    return pts


def host_constants(K):
    """Pre-centered monomials + packed tail constants (f32)."""
    pts = grid_pts(K)
    x, y = pts[:, 0], pts[:, 1]
    c0x = np.float32(x.mean())
    c0y = np.float32(y.mean())
    xs = (x - c0x).astype(np.float32)
    ys = (y - c0y).astype(np.float32)
    Mp = np.stack([xs * xs, xs * ys, xs, ys * ys, ys, np.ones_like(xs)],
                  -1).astype(np.float32)

    cpack = np.zeros((9, C_TOT), np.float32)
    cpack[:9, C_I9H:C_I9H + 9] = 0.5 * np.eye(9, dtype=np.float32)
    pid = _pidx()
    for a in range(3):
        for b in range(3):
            cpack[pid[(a, b)], C_ET69 + 3 * a + b] = 1.0
    cpack[:3, C_I3:C_I3 + 3] = np.eye(3, dtype=np.float32)
    cpack[:9, C_V09] = 1.0 / 3.0
    cpack[:6, C_V06] = INV_SQRT3
    cpack[:3, C_SEL1:C_SEL1 + 3] = np.eye(3, dtype=np.float32)
    cpack[:3, C_SEL2 + 3:C_SEL2 + 6] = np.eye(3, dtype=np.float32)

    # Sh(dx,dy) = I + dx*E1 + dy*E2 + dx^2*E3 + dx*dy*E4 + dy^2*E5
    # (rows of L before the diag scale; see proto.Lmat)
    E1 = np.zeros((6, 6), np.float32)  # dx terms
    E1[0, 2] = -2.0
    E1[1, 4] = -1.0
    E1[2, 5] = -1.0
    E2 = np.zeros((6, 6), np.float32)  # dy terms
    E2[1, 2] = -1.0
    E2[3, 4] = -2.0
    E2[4, 5] = -1.0
    E3 = np.zeros((6, 6), np.float32)  # dx^2
    E3[0, 5] = 1.0
    E4 = np.zeros((6, 6), np.float32)  # dx*dy
    E4[1, 5] = 1.0
    E5 = np.zeros((6, 6), np.float32)  # dy^2
    E5[3, 5] = 1.0
    mats = [np.eye(6, dtype=np.float32), E1, E2, E3, E4, E5]
    for i, Em in enumerate(mats):
        cpack[:6, C_SHT + 6 * i:C_SHT + 6 * i + 6] = Em.T
    # svec masks: svec = [s2,s2,s,s2,s,1] = c2m*s2 + c1m*s + c0m
    cpack[:6, C_MSK + 0] = [1, 1, 0, 1, 0, 0]
    cpack[:6, C_MSK + 1] = [0, 0, 1, 0, 1, 0]
    cpack[:6, C_MSK + 2] = [0, 0, 0, 0, 0, 1]
    cpack[:9, C_IDN:C_IDN + 9] = np.eye(9, dtype=np.float32)
    cpack[:9, C_ONE:C_ONE + 9] = 1.0
    return Mp, cpack, float(c0x), float(c0y)


def _tile128(a, ntiles):
    """[ntiles*128, F] -> [128, ntiles*F] with [p, t*F+f] = a[t*128+p, f]."""
    F = a.shape[1]
    return np.ascontiguousarray(
        a.reshape(ntiles, 128, F).transpose(1, 0, 2).reshape(128, ntiles * F)
    )


DEFAULT_K = np.array([[500.0, 0.0, 320.0], [0.0, 500.0, 240.0],
                      [0.0, 0.0, 1.0]], np.float32)


def build_nc(repeats=1, no_coll=False, no_tail=False, use_f32r=True,
             dbg_c=False, c0=None):
    if c0 is None:
        _, _, c0x_, c0y_ = host_constants(DEFAULT_K)
        c0 = (c0x_, c0y_)
    nc = bacc.Bacc("TRN2", target_bir_lowering=False, debug=False,
                   num_devices=NCORES)

    xn = nc.dram_tensor("xn", [128, RT * N], F32, kind="ExternalInput")
    xc = nc.dram_tensor("xc", [128, RT * N], F32, kind="ExternalInput")
    m1f = nc.dram_tensor("m1f", [128, CB * 6], F32, kind="ExternalInput")
    m2s = nc.dram_tensor("m2s", [128, RT * 6], F32, kind="ExternalInput")
    cpk = nc.dram_tensor("cpack", [9, C_TOT], F32, kind="ExternalInput")
    out_d = nc.dram_tensor("out", [6, 6] if dbg_c else [3, 3], F32, kind="ExternalOutput")

    tr_in = nc.dram_tensor("tr_in", [1, SH], F32)
    tr_out = nc.dram_tensor("tr_out", [NCORES, SH], F32, addr_space="Shared")
    cr_in = nc.dram_tensor("cr_in", [6, 6], F32)
    cr_out = nc.dram_tensor("cr_out", [6, 6], F32, addr_space="Shared")
    stage = nc.dram_tensor("stage", [64], F32)
    mshuf = nc.dram_tensor("mshuf", [81], F32)

    groups = [list(range(NCORES))]

    with tile.TileContext(nc) as tc:
        with (
            tc.tile_pool(name="persist", bufs=1) as pp,
            tc.tile_pool(name="scratch", bufs=2) as sp,
            tc.tile_pool(name="ps_t", bufs=2, space="PSUM") as ps,
            tc.tile_pool(name="ps_T", bufs=2, space="PSUM") as psT,
            tc.tile_pool(name="ps_c", bufs=1, space="PSUM") as psc,
        ):
            for _rep in range(repeats):
                # ---------- P0: loads (XN on qSP, XC on qACT) ----------
                XN = pp.tile([128, RT * N], F32, tag="XN")
                XC = pp.tile([128, RT * N], F32, tag="XC")
                HN = N // 2
                for t in range(RT):
                    a = t * N
                    nc.sync.dma_start(XN[:, a:a + HN], xn[:, a:a + HN])
                    nc.scalar.dma_start(XN[:, a + HN:a + N],
                                        xn[:, a + HN:a + N])
                for t in range(RT):
                    a = t * N
                    nc.sync.dma_start(XC[:, a:a + HN], xc[:, a:a + HN])
                    nc.scalar.dma_start(XC[:, a + HN:a + N],
                                        xc[:, a + HN:a + N])
                m1s_s = pp.tile([128, CB * 6], F32, tag="m1f")
                nc.scalar.dma_start(m1s_s[:], m1f[:])
                m2s_s = pp.tile([128, RT * 6], F32, tag="m2s")
                nc.scalar.dma_start(m2s_s[:], m2s[:])
                cps = pp.tile([9, C_TOT], F32, tag="cpk")
                nc.scalar.dma_start(cps[:], cpk[:])
                sqwarm = sp.tile([1, 1], F32, tag="sqwarm")
                nc.scalar.activation(sqwarm[:], cps[0:1, 0:1], AF.Sqrt)

                def XNt(t):
                    return XN[:, t * N:(t + 1) * N]

                def XCt(t):
                    return XC[:, t * N:(t + 1) * N]

                # ---------- P1: row thresholds -> coll1 ----------
                r8 = pp.tile([128, RT * 8], F32, tag="r8")
                for t in range(RT):
                    nc.vector.max(out=r8[:, t * 8:t * 8 + 8], in_=XNt(t))
                trT0 = pp.tile([128, RT], F32, tag="trT0")
                nc.vector.tensor_scalar_max(
                    trT0[:],
                    r8[:].rearrange("p (t e) -> p t e", e=8)[:, :, 2], T0)
                for t in range(RT):
                    nc.sync.dma_start(tr_in[0:1, t * 128:(t + 1) * 128],
                                      trT0[:, t:t + 1])

                if no_coll:
                    nc.sync.dma_start(tr_out[0:1, :], tr_in[:])
                else:
                    nc.gpsimd.collective_compute(
                        "AllGather", OP.bypass, replica_groups=groups,
                        ins=[tr_in[:]], outs=[tr_out[:]])

                # ---------- P2: col thresholds (local, exact) ----------
                c8 = pp.tile([128, RT * 8], F32, tag="c8")
                for t in range(RT):
                    nc.vector.max(out=c8[:, t * 8:t * 8 + 8], in_=XCt(t))

                # ---------- P3: broadcast row-threshold table ----------
                trow = pp.tile([1, N], F32, tag="trow")
                nc.sync.dma_start(trow[:], tr_out[:].rearrange("k i -> (k i)"))
                trB = pp.tile([128, N], F32, tag="trB")
                MCH = 1536
                for c0_, cw in ((0, 768), (768, 768), (1536, 1536)):
                    nc.gpsimd.partition_broadcast(
                        trB[:, c0_:c0_ + cw], trow[:, c0_:c0_ + cw],
                        channels=128)

                # ---------- P4: dense mask + fp32r T-gram ----------
                # W (f32r): W = XC * [XC >= max(trB, tc_t)]
                # T[b, r] = sum_c m2'[c, b] * W^T[c, r]   (PSUM chunks [6,512])
                WDT = F32R if use_f32r else F32
                m2r = pp.tile([128, RT * 6], WDT, tag="m2r")
                nc.vector.tensor_copy(m2r[:], m2s_s[:])
                Wr = pp.tile([128, RT * N], WDT, tag="Wr")
                Wf = pp.tile([128, N], F32, tag="Wf")  # t=2 chunk via gpsimd
                Tsb = pp.tile([6, N], F32, tag="Tsb")
                TT = pp.tile([128, CB * 6], F32, tag="TT")
                i6 = cps[0:6, C_IDN:C_IDN + 6]
                for h in range(2):
                    for t in range(RT):
                        tcl = c8[:, t * 8 + 2:t * 8 + 3]
                        sl = slice(t * N + h * MCH, t * N + (h + 1) * MCH)
                        msk = pp.tile([128, MCH], F32, tag=f"msk{h}{t}")
                        nc.vector.scalar_tensor_tensor(
                            msk[:], trB[:, h * MCH:(h + 1) * MCH], tcl,
                            XC[:, sl], OP.max, OP.is_le)
                        if t == 0:
                            nc.gpsimd.tensor_tensor(
                                Wf[:, h * MCH:(h + 1) * MCH], XC[:, sl],
                                msk[:], OP.mult)
                        else:
                            nc.vector.tensor_tensor(Wr[:, sl], XC[:, sl],
                                                    msk[:], OP.mult)
                    for q in range(3):
                        ch = h * 3 + q
                        Tp = psT.tile([6, 512], F32, tag="Tp")
                        for t in range(RT):
                            c0_ = t * N + h * MCH + q * 512
                            if t == 0:
                                nc.tensor.matmul(
                                    Tp[:], m2s_s[:, t * 6:(t + 1) * 6],
                                    Wf[:, h * MCH + q * 512:
                                        h * MCH + q * 512 + 512],
                                    start=True, stop=False)
                            else:
                                nc.tensor.matmul(
                                    Tp[:],
                                    m2r[:, t * 6:(t + 1) * 6],
                                    Wr[:, c0_:c0_ + 512],
                                    start=False, stop=(t == RT - 1))
                        nc.scalar.activation(Tsb[:, ch * 512:(ch + 1) * 512],
                                             Tp[:], AF.Copy)
                        # PE-transpose T chunk into TT[p, (j b)] blocks
                        for jj in range(4):
                            j = ch * 4 + jj
                            pt = ps.tile([128, 6], F32, tag="tps")
                            nc.tensor.transpose(
                                pt[:], Tsb[:, j * 128:(j + 1) * 128], i6)
                            nc.scalar.activation(TT[:, j * 6:(j + 1) * 6],
                                                 pt[:], AF.Copy)

                # C[a, b] = sum_j m1'_j^T TT_j  (two groups for overlap)
                pc0 = psc.tile([6, 6], F32, tag="pc0")
                pc1 = psc.tile([6, 6], F32, tag="pc1")
                for j in range(CB):
                    pc = pc0 if j < 12 else pc1
                    nc.tensor.matmul(pc[:], m1s_s[:, j * 6:(j + 1) * 6],
                                     TT[:, j * 6:(j + 1) * 6],
                                     start=(j % 12 == 0), stop=(j % 12 == 11))
                Cp = sp.tile([6, 6], F32, tag="Cp")
                nc.vector.tensor_copy(Cp[:], pc0[:])
                nc.vector.tensor_tensor(Cp[:], Cp[:], pc1[:], OP.add)
                nc.sync.dma_start(cr_in[:], Cp[:])

                # ---------- coll2: AllReduce 6x6 gram ----------
                if no_coll:
                    nc.sync.dma_start(cr_out[:], cr_in[:])
                else:
                    nc.gpsimd.collective_compute(
                        "AllReduce", OP.add, replica_groups=groups,
                        ins=[cr_in[:]], outs=[cr_out[:]])

                if no_tail:
                    nn = 6 if dbg_c else 3
                    dummy = sp.tile([nn, nn], F32, tag="dummy")
                    nc.sync.dma_start(dummy[:], cr_out[0:nn, 0:nn])
                    nc.sync.dma_start(out_d[:], dummy[:])
                    continue

                # ---------- tail ----------
                _tail(nc, pp, sp, ps, psc, cps, c0, cr_out, stage, mshuf, out_d)

    nc.compile()
    return nc


def _transpose(nc, ps, sp, in_sb, n, idn, tag):
    pt = ps.tile([n, n], F32, tag="tps")
    nc.tensor.transpose(pt[:], in_sb, idn[:n, :n])
    ot = sp.tile([n, n], F32, tag=f"ot_{tag}")
    nc.vector.tensor_copy(ot[:], pt[:])
    return ot


def _powchain(nc, ps, sp, m_sb, n, tag, n_squarings=5, extra=True):
    """M^50 (extra=True: 5 squarings + M48=M32@M16 + M50=M48@M2) or M^32."""
    powers = {}
    cur = m_sb
    for i in range(1, n_squarings + 1):
        pm = ps.tile([n, n], F32, tag="tps")
        nc.tensor.matmul(pm[:], cur, cur, start=True, stop=True)
        nxt = sp.tile([n, n], F32, tag=f"pw_{tag}_{i}")
        nc.vector.tensor_scalar_mul(nxt[:], pm[:], 2.0)
        powers[2 ** i] = nxt
        cur = nxt[:]
    if not extra:
        return powers[2 ** n_squarings]
    pm = ps.tile([n, n], F32, tag="tps")
    nc.tensor.matmul(pm[:], powers[32][:], powers[16][:], start=True, stop=True)
    m48 = sp.tile([n, n], F32, tag=f"pw_{tag}_48")
    nc.vector.tensor_scalar_mul(m48[:], pm[:], 2.0)
    pm = ps.tile([n, n], F32, tag="tps")
    nc.tensor.matmul(pm[:], m48[:], powers[2][:], start=True, stop=True)
    m50 = sp.tile([n, n], F32, tag=f"pw_{tag}_50")
    nc.vector.tensor_scalar_mul(m50[:], pm[:], 2.0)
    return m50


def _tail(nc, pp, sp, ps, psc, cps, c0, cr_out, stage, mshuf, out_d):
    """C' -> Hartley -> L-transform -> Mmat -> chains -> projection."""
    idn = cps[0:9, C_IDN:C_IDN + 9]

    Cp = sp.tile([6, 6], F32, tag="Cpr")
    nc.sync.dma_start(Cp[:], cr_out[:])
    CpT = sp.tile([6, 6], F32, tag="CprT")
    nc.scalar.dma_start(CpT[:], cr_out[:].rearrange("a b -> b a"))

    # moments [1,12]: side1 = C'[:,5], side2 = C'[5,:] (parallel queues)
    sc = pp.tile([128, 112], F32, tag="tailsc")
    nc.scalar.dma_start(sc[0:1, 0:6],
                        cr_out[:].rearrange("a b -> b a")[5:6, :])
    nc.sync.dma_start(sc[0:1, 6:12], cr_out[5:6, :])

    def scv(a, b):
        return sc[0:1, a:b]

    def pair(k):
        return sc[0:1, 0:12].rearrange("p (g d) -> p d g", g=2)[:, k, :]

    Sxx, Sx, Syy, Sy, Sw = pair(0), pair(2), pair(3), pair(4), pair(5)
    ws = scv(12, 14); nc.vector.tensor_scalar_add(ws, Sw, EPS)
    rws = scv(14, 16); nc.vector.reciprocal(rws, ws)
    cx = scv(16, 18); nc.vector.tensor_tensor(cx, Sx, rws, OP.mult)  # = dx
    cy = scv(18, 20); nc.vector.tensor_tensor(cy, Sy, rws, OP.mult)  # = dy
    t_a = scv(20, 22); nc.vector.tensor_tensor(t_a, cx, Sx, OP.mult)
    t_b = scv(22, 24); nc.vector.tensor_tensor(t_b, cy, Sy, OP.mult)
    cdS = scv(24, 26); nc.vector.tensor_tensor(cdS, t_a, t_b, OP.add)
    u_a = scv(26, 28); nc.vector.tensor_tensor(u_a, cx, cx, OP.mult)
    u_b = scv(28, 30); nc.vector.tensor_tensor(u_b, cy, cy, OP.mult)
    c2_ = scv(30, 32); nc.vector.tensor_tensor(c2_, u_a, u_b, OP.add)
    sq_ = scv(32, 34); nc.vector.tensor_tensor(sq_, Sxx, Syy, OP.add)
    n2c = scv(34, 36); nc.vector.tensor_scalar_mul(n2c, cdS, -2.0)
    c2w = scv(36, 38); nc.vector.tensor_tensor(c2w, c2_, Sw, OP.mult)
    m_ = scv(38, 40); nc.vector.tensor_tensor(m_, sq_, n2c, OP.add)
    m2_ = scv(40, 42); nc.vector.tensor_tensor(m2_, m_, c2w, OP.add)
    md2 = scv(42, 44); nc.vector.tensor_tensor(md2, m2_, rws, OP.mult)
    md2e = scv(44, 46); nc.vector.tensor_scalar_add(md2e, md2, EPS)
    md = scv(46, 48); nc.scalar.activation(md, md2e, AF.Sqrt)
    mde = scv(48, 50); nc.vector.tensor_scalar_add(mde, md, EPS)
    rmd = scv(50, 52); nc.vector.reciprocal(rmd, mde)
    s_ = scv(52, 54); nc.vector.tensor_scalar_mul(s_, rmd, SQRT2)
    # real centroids: cr = dx + c0 ; c0s = [c0x c0x c0y c0y] paired
    cxr = scv(54, 56); nc.vector.tensor_scalar_add(cxr, cx, c0[0])
    cyr = scv(56, 58); nc.vector.tensor_scalar_add(cyr, cy, c0[1])
    scx = scv(58, 60); nc.vector.tensor_tensor(scx, s_, cxr, OP.mult)
    scy = scv(60, 62); nc.vector.tensor_tensor(scy, s_, cyr, OP.mult)
    nscx = scv(62, 64); nc.vector.tensor_scalar_mul(nscx, scx, -1.0)
    nscy = scv(64, 66); nc.vector.tensor_scalar_mul(nscy, scy, -1.0)
    # L scalars: s2, dx2, dxy, dy2 (paired)
    s2p = scv(66, 68); nc.vector.tensor_tensor(s2p, s_, s_, OP.mult)
    dx2 = scv(68, 70); nc.vector.tensor_tensor(dx2, cx, cx, OP.mult)
    dxy = scv(70, 72); nc.vector.tensor_tensor(dxy, cx, cy, OP.mult)
    dy2 = scv(72, 74); nc.vector.tensor_tensor(dy2, cy, cy, OP.mult)

    # T row-major 9-vectors: t1v at 76:85, t2v at 85:94
    nc.vector.memset(scv(76, 94), 0.0)
    tv = sc[0:1, 76:94]
    tv9 = tv.rearrange("p (v f) -> p v f", v=2)
    nc.vector.tensor_copy(tv9[:, :, 0:1], s_.unsqueeze(2))
    nc.vector.tensor_copy(tv9[:, :, 4:5], s_.unsqueeze(2))
    nc.vector.tensor_copy(
        tv9[:, :, 2:8].rearrange("p v (c d) -> p v c d", c=2)[:, :, :, 0:1],
        sc[0:1, 62:66].rearrange("p (c v) -> p v c", c=2).unsqueeze(3))
    nc.vector.memset(tv9[:, :, 8:9], 1.0)
    nc.sync.dma_start(stage[0:18], tv)
    T12 = sp.tile([3, 6], F32, tag="T12")
    nc.sync.dma_start(
        T12[:].rearrange("i (v j) -> i v j", v=2),
        stage[0:18].rearrange("(v i j) -> i v j", i=3, j=3))

    # broadcast scalar strip to 6 partitions for the L build (PE ones);
    # consumers read the PSUM bank directly
    ones16 = cps[0:1, C_ONE:C_ONE + 6]
    scBt = psc.tile([6, 80], F32, tag="scBp")
    nc.tensor.matmul(scBt[:], ones16, sc[0:1, 0:80], start=True, stop=True)
    scB = scBt

    def shT(side, tag):
        """Sh^T for side (0/1): I^T + dx E1^T + dy E2^T + dx2 E3^T + ..."""
        dx = scB[:, 16 + side:17 + side]
        dy = scB[:, 18 + side:19 + side]
        dx2_ = scB[:, 68 + side:69 + side]
        dxy_ = scB[:, 70 + side:71 + side]
        dy2_ = scB[:, 72 + side:73 + side]
        def M(i):
            return cps[0:6, C_SHT + 6 * i:C_SHT + 6 * i + 6]
        acc = sp.tile([6, 6], F32, tag=f"sh_{tag}")
        nc.vector.scalar_tensor_tensor(acc[:], M(1), dx, M(0), OP.mult, OP.add)
        for i, sval in [(2, dy), (3, dx2_), (4, dxy_), (5, dy2_)]:
            nc.vector.scalar_tensor_tensor(acc[:], M(i), sval, acc[:],
                                           OP.mult, OP.add)
        return acc

    Sh1T = shT(0, "1")
    Sh2T = shT(1, "2")
    # svec side1 as a [6,1] column (per-partition): c2m*s2 + c1m*s + c0m
    sv1c = sp.tile([6, 1], F32, tag="sv1c")
    tmp1 = sp.tile([6, 1], F32, tag="svt1")
    nc.vector.scalar_tensor_tensor(
        tmp1[:], cps[0:6, C_MSK:C_MSK + 1], scB[:, 66:67],
        cps[0:6, C_MSK + 2:C_MSK + 3], OP.mult, OP.add)
    nc.vector.scalar_tensor_tensor(
        sv1c[:], cps[0:6, C_MSK + 1:C_MSK + 2], scB[:, 52:53],
        tmp1[:], OP.mult, OP.add)
    # svec side2 as a [1,6] row on partition 0: [s2 s2 s s2 s 1]
    svr2 = sc[0:1, 96:102]
    s2v2 = sc[0:1, 67:68]
    sv2 = sc[0:1, 53:54]
    nc.vector.tensor_copy(
        svr2.rearrange("p (a b) -> p a b", a=3)[:, 0:2, 0:1],
        s2v2.unsqueeze(2).to_broadcast([1, 2, 1]))   # slots 0,2 = s2 (a-major)
    nc.vector.tensor_copy(svr2[:, 1:2], s2v2)        # slot 1 = s2
    nc.vector.tensor_copy(svr2[:, 3:4], s2v2)        # slot 3 = s2
    nc.vector.tensor_copy(svr2[:, 2:3], sv2)         # slot 2 = s
    nc.vector.tensor_copy(svr2[:, 4:5], sv2)         # slot 4 = s
    nc.vector.memset(svr2[:, 5:6], 1.0)
    sv2B = sp.tile([6, 6], F32, tag="sv2B")
    sv2Bp = ps.tile([6, 6], F32, tag="tps")
    nc.tensor.matmul(sv2Bp[:], ones16, svr2, start=True, stop=True)
    nc.vector.tensor_copy(sv2B[:], sv2Bp[:])

    # C2 = D1 Sh1 C' Sh2^T D2
    vps = ps.tile([6, 6], F32, tag="tps")
    nc.tensor.matmul(vps[:], Sh1T[:], Cp[:], start=True, stop=True)  # Sh1 C'
    vS = sp.tile([6, 6], F32, tag="vS")
    nc.vector.tensor_copy(vS[:], vps[:])
    vT = _transpose(nc, ps, sp, vS[:], 6, idn, "vT")
    ups = ps.tile([6, 6], F32, tag="tps")
    nc.tensor.matmul(ups[:], vT[:], Sh2T[:], start=True, stop=True)  # v Sh2^T
    # C2[r, c] = svec1[r] * u[r, c] * svec2[c]
    u1 = sp.tile([6, 6], F32, tag="u1")
    nc.vector.tensor_scalar_mul(u1[:], ups[:], sv1c[:])
    C2 = sp.tile([6, 6], F32, tag="C2")
    nc.vector.tensor_tensor(C2[:], u1[:], sv2B[:], OP.mult)
    C2T = _transpose(nc, ps, sp, C2[:], 6, idn, "c2t")

    _solve(nc, pp, sp, ps, psc, cps, idn, sc, C2[:], C2T[:], stage, mshuf,
           out_d, T12)


def _solve(nc, pp, sp, ps, psc, cps, idn, sc, C2, C2T, stage, mshuf, out_d,
           T12):
    i9h = cps[0:9, C_I9H:C_I9H + 9]
    et69 = cps[0:6, C_ET69:C_ET69 + 9]
    i3c = cps[0:3, C_I3:C_I3 + 3]
    v09 = cps[0:9, C_V09:C_V09 + 1]
    v06 = cps[0:6, C_V06:C_V06 + 1]
    sel1 = cps[0:3, C_SEL1:C_SEL1 + 6]
    sel2 = cps[0:3, C_SEL2:C_SEL2 + 6]

    # G2 = E C2 E^T : G2[3a+b, 3c+d] = C2[pair(a,b), pair(c,d)]
    z_ps = ps.tile([6, 9], F32, tag="tps")
    nc.tensor.matmul(z_ps[:], C2T, et69, start=True, stop=True)  # C2 E^T
    Zs = sp.tile([6, 9], F32, tag="Zs")
    nc.vector.tensor_copy(Zs[:], z_ps[:])
    g_ps = ps.tile([9, 9], F32, tag="tps")
    nc.tensor.matmul(g_ps[:], et69, Zs[:], start=True, stop=True)    # E @ Z
    G2 = sp.tile([9, 9], F32, tag="G2")
    nc.vector.tensor_copy(G2[:], g_ps[:])

    # Mmat[3p+q, 3r+s] = G2[3p+r, 3q+s]: bounce via DRAM
    nc.sync.dma_start(mshuf[:], G2[:])
    Mmat = sp.tile([9, 9], F32, tag="Mmat")
    for p in range(3):
        eng = nc.scalar if p == 1 else nc.sync
        eng.dma_start(
            Mmat[3 * p:3 * p + 3, :].rearrange("q (r s) -> q r s", s=3),
            mshuf[:].rearrange("(p q1 r s) -> p q1 r s", p=3, q1=3, r=3)
            .transpose([0, 2, 1, 3])[p])

    # Msp = Mmat/(2 lam) - I/2
    dg = sp.tile([9, 9], F32, tag="dg")
    nc.vector.tensor_tensor(dg[:], Mmat[:], i9h, OP.mult)
    lam2 = sp.tile([9, 1], F32, tag="lam2")
    nc.vector.tensor_reduce(lam2[:], dg[:], AX.X, OP.add)
    ones99 = cps[0:9, C_ONE:C_ONE + 9]
    lam2r = ps.tile([9, 1], F32, tag="tps")
    nc.tensor.matmul(lam2r[:], ones99, lam2[:], start=True, stop=True)
    lam4 = sp.tile([9, 1], F32, tag="lam4")
    nc.vector.tensor_scalar_mul(lam4[:], lam2r[:], 4.0)
    inv2l = sp.tile([9, 1], F32, tag="inv2l")
    nc.vector.reciprocal(inv2l[:], lam4[:])
    Msp = sp.tile([9, 9], F32, tag="Msp")
    nc.vector.scalar_tensor_tensor(Msp[:], Mmat[:], inv2l[:], i9h,
                                   OP.mult, OP.subtract)
    M50 = _powchain(nc, ps, sp, Msp[:], 9, "m9", 5, extra=True)

    w9ps = ps.tile([1, 9], F32, tag="tps")
    nc.tensor.matmul(w9ps[:], v09, M50[:], start=True, stop=True)
    w9 = sp.tile([1, 9], F32, tag="w9")
    nc.vector.tensor_copy(w9[:], w9ps[:])
    nc.sync.dma_start(stage[24:33], w9[:])  # raw; 1/||w9|| folded at the end
    w9sq = sp.tile([1, 9], F32, tag="w9sq")
    nc.vector.tensor_tensor(w9sq[:], w9[:], w9[:], OP.mult)
    nn9 = sp.tile([1, 1], F32, tag="nn9")
    nc.vector.tensor_reduce(nn9[:], w9sq[:], AX.X, OP.add)
    sr9 = sp.tile([1, 1], F32, tag="sr9")
    nc.scalar.activation(sr9[:], nn9[:], AF.Sqrt)
    rs9 = sp.tile([1, 1], F32, tag="rs9")
    nc.vector.reciprocal(rs9[:], sr9[:])
    rs9c = psc.tile([3, 1], F32, tag="rs9c")
    nc.tensor.matmul(rs9c[:], cps[0:1, C_ONE:C_ONE + 3], rs9[:],
                     start=True, stop=True)

    # E = T2^T E_raw T1 (and E^T);  T1m/T2m preloaded in T12
    T1m = T12[:, 0:3]
    T2m = T12[:, 3:6]
    Eraw = sp.tile([3, 3], F32, tag="Eraw")
    nc.sync.dma_start(Eraw[:], stage[24:33].rearrange("(i j) -> i j", j=3))

    a1ps = ps.tile([3, 3], F32, tag="tps")
    nc.tensor.matmul(a1ps[:], T2m, Eraw[:], start=True, stop=True)
    A1 = sp.tile([3, 3], F32, tag="A1")
    nc.vector.tensor_copy(A1[:], a1ps[:])
    A1T = _transpose(nc, ps, sp, A1[:], 3, idn, "a1t")
    etps = ps.tile([3, 3], F32, tag="tps")
    nc.tensor.matmul(etps[:], T1m, A1T[:], start=True, stop=True)
    ETs = sp.tile([3, 3], F32, tag="ETs")
    nc.vector.tensor_copy(ETs[:], etps[:])
    Es = _transpose(nc, ps, sp, ETs[:], 3, idn, "es")

    # B = E^T E ; blockdiag 6x6 chain (32 iters) for v1 (max) and v3 (min)
    bps = ps.tile([3, 3], F32, tag="tps")
    nc.tensor.matmul(bps[:], Es[:], Es[:], start=True, stop=True)
    Bm = sp.tile([3, 3], F32, tag="Bm")
    nc.vector.tensor_copy(Bm[:], bps[:])
    dg3 = sp.tile([3, 3], F32, tag="dg3")
    nc.vector.tensor_tensor(dg3[:], Bm[:], i3c, OP.mult)
    lb = sp.tile([3, 1], F32, tag="lb")
    nc.vector.tensor_reduce(lb[:], dg3[:], AX.X, OP.add)
    lbr = ps.tile([3, 1], F32, tag="tps")
    nc.tensor.matmul(lbr[:], cps[0:3, C_ONE:C_ONE + 3], lb[:],
                     start=True, stop=True)
    invlb = sp.tile([3, 1], F32, tag="invlb")
    nc.vector.reciprocal(invlb[:], lbr[:])
    Bs3 = sp.tile([3, 3], F32, tag="Bs3")
    nc.vector.tensor_scalar_mul(Bs3[:], Bm[:], invlb[:])
    IB = sp.tile([3, 3], F32, tag="IB")
    nc.vector.tensor_tensor(IB[:], i3c, Bs3[:], OP.subtract)
    bdps = ps.tile([6, 6], F32, tag="tps")
    nc.tensor.matmul(bdps[:, 0:3], sel1, Bs3[:], start=True, stop=True)
    nc.tensor.matmul(bdps[:, 3:6], sel2, IB[:], start=True, stop=True)
    BD = sp.tile([6, 6], F32, tag="BD")
    nc.vector.tensor_copy(BD[:], bdps[:])
    BD32 = _powchain(nc, ps, sp, BD[:], 6, "m6", 5, extra=False)

    w6ps = ps.tile([1, 6], F32, tag="tps")
    nc.tensor.matmul(w6ps[:], v06, BD32[:], start=True, stop=True)
    w6 = sp.tile([1, 6], F32, tag="w6")
    nc.vector.tensor_copy(w6[:], w6ps[:])
    w6sq = sp.tile([1, 6], F32, tag="w6sq")
    nc.vector.tensor_tensor(w6sq[:], w6[:], w6[:], OP.mult)
    nn6 = sp.tile([1, 2], F32, tag="nn6")
    nc.vector.tensor_reduce(nn6[:].unsqueeze(2),
                            w6sq[:].rearrange("p (g d) -> p g d", g=2), AX.X,
                            OP.add)
    sr6 = sp.tile([1, 2], F32, tag="sr6")
    nc.scalar.activation(sr6[:], nn6[:], AF.Sqrt)
    rs6 = sp.tile([1, 2], F32, tag="rs6")
    nc.vector.reciprocal(rs6[:], sr6[:])
    vv = sp.tile([1, 6], F32, tag="vv")
    nc.vector.tensor_tensor(
        vv[:].rearrange("p (g d) -> p g d", g=2),
        w6[:].rearrange("p (g d) -> p g d", g=2),
        rs6[:].unsqueeze(2).to_broadcast([1, 2, 3]), OP.mult)

    # v2 = cross(v3, v1), normalized with EPS
    aa = sp.tile([1, 6], F32, tag="aa")
    nc.vector.tensor_copy(
        aa[:].rearrange("p (r d) -> p r d", r=2),
        vv[:, 3:6].unsqueeze(1).to_broadcast([1, 2, 3]))
    bb = sp.tile([1, 6], F32, tag="bb")
    nc.vector.tensor_copy(
        bb[:].rearrange("p (r d) -> p r d", r=2),
        vv[:, 0:3].unsqueeze(1).to_broadcast([1, 2, 3]))
    cr1 = sp.tile([1, 3], F32, tag="cr1")
    nc.vector.tensor_tensor(cr1[:], aa[:, 1:4], bb[:, 2:5], OP.mult)
    cr2 = sp.tile([1, 3], F32, tag="cr2")
    nc.vector.tensor_tensor(cr2[:], aa[:, 2:5], bb[:, 1:4], OP.mult)
    v2r = sp.tile([1, 3], F32, tag="v2r")
    nc.vector.tensor_tensor(v2r[:], cr1[:], cr2[:], OP.subtract)
    v2sq = sp.tile([1, 3], F32, tag="v2sq")
    nc.vector.tensor_tensor(v2sq[:], v2r[:], v2r[:], OP.mult)
    nn2 = sp.tile([1, 1], F32, tag="nn2")
    nc.vector.tensor_reduce(nn2[:], v2sq[:], AX.X, OP.add)
    sr2 = sp.tile([1, 1], F32, tag="sr2")
    nc.scalar.activation(sr2[:], nn2[:], AF.Sqrt)
    sr2e = sp.tile([1, 1], F32, tag="sr2e")
    nc.vector.tensor_scalar_add(sr2e[:], sr2[:], EPS)
    rs2 = sp.tile([1, 1], F32, tag="rs2")
    nc.vector.reciprocal(rs2[:], sr2e[:])
    v2 = sp.tile([1, 3], F32, tag="v2")
    nc.vector.tensor_tensor(v2[:], v2r[:], rs2[:].to_broadcast([1, 3]), OP.mult)

    vvv = sp.tile([1, 6], F32, tag="vvv")
    nc.vector.tensor_copy(vvv[:, 0:3], vv[:, 0:3])
    nc.vector.tensor_copy(vvv[:, 3:6], v2[:])
    nc.sync.dma_start(stage[33:39], vvv[:])
    Vr = sp.tile([2, 3], F32, tag="Vr")
    nc.sync.dma_start(Vr[:], stage[33:39].rearrange("(i k) -> i k", k=3))
    Vc = sp.tile([3, 2], F32, tag="Vc")
    nc.scalar.dma_start(Vc[:], stage[33:39].rearrange("(i k) -> k i", k=3))
    evps = ps.tile([2, 3], F32, tag="tps")
    nc.tensor.matmul(evps[:], Vc[:], ETs[:], start=True, stop=True)
    Evr = sp.tile([2, 3], F32, tag="Evr")
    nc.vector.tensor_copy(Evr[:], evps[:])
    evsq = sp.tile([2, 3], F32, tag="evsq")
    nc.vector.tensor_tensor(evsq[:], Evr[:], Evr[:], OP.mult)
    ss2 = sp.tile([2, 1], F32, tag="ss2")
    nc.vector.tensor_reduce(ss2[:], evsq[:], AX.X, OP.add)
    sv = sp.tile([2, 1], F32, tag="sv")
    nc.scalar.activation(sv[:], ss2[:], AF.Sqrt)
    ssum = ps.tile([2, 1], F32, tag="tps")
    nc.tensor.matmul(ssum[:], cps[0:2, C_ONE:C_ONE + 2], sv[:],
                     start=True, stop=True)
    savg = sp.tile([2, 1], F32, tag="savg")
    nc.vector.tensor_scalar_mul(savg[:], ssum[:], 0.5)
    sve = sp.tile([2, 1], F32, tag="sve")
    nc.vector.tensor_scalar_add(sve[:], sv[:], EPS)
    rsv = sp.tile([2, 1], F32, tag="rsv")
    nc.vector.reciprocal(rsv[:], sve[:])
    f2 = sp.tile([2, 1], F32, tag="f2")
    nc.vector.tensor_tensor(f2[:], rsv[:], savg[:], OP.mult)
    U2 = sp.tile([2, 3], F32, tag="U2")
    nc.vector.tensor_scalar_mul(U2[:], Evr[:], f2[:])
    ops_ = ps.tile([3, 3], F32, tag="tps")
    nc.tensor.matmul(ops_[:], U2[:], Vr[:], start=True, stop=True)
    outs = sp.tile([3, 3], F32, tag="outs")
    nc.vector.tensor_scalar_mul(outs[:], ops_[:], rs9c[:])
    nc.sync.dma_start(out_d[:], outs[:])


def make_in_maps(P, K):
    P = np.asarray(P, np.float32)
    K = np.asarray(K, np.float32)
    Pc = np.ascontiguousarray(P[:N, :N])
    PcT = np.ascontiguousarray(Pc.T)
    Mp, cpack, c0x, c0y = host_constants(K)
    m1full = _tile128(Mp, CB)
    in_maps = []
    for k in range(NCORES):
        in_maps.append({
            "xn": _tile128(Pc[k * SH:(k + 1) * SH], RT),
            "xc": _tile128(PcT[k * SH:(k + 1) * SH], RT),
            "m1f": m1full,
            "m2s": _tile128(Mp[k * SH:(k + 1) * SH], RT),
            "cpack": cpack,
        })
    return in_maps


_NC_CACHE = {}


def kernel(P, K):
    from concourse.bass_utils import run_bass_kernel_spmd
    if "nc" not in _NC_CACHE:
        _, _, c0x, c0y = host_constants(np.asarray(K, np.float32))
        _NC_CACHE["nc"] = build_nc(c0=(c0x, c0y))
    nc = _NC_CACHE["nc"]
    in_maps = make_in_maps(P, K)
    res = run_bass_kernel_spmd(nc, in_maps, core_ids=list(range(NCORES)))
    return np.asarray(res.results[0]["out"], np.float32)
